# revision 44
# baseline (speedup 1.0000x reference)
"""GAT-style attention-diagonal kernel for Trainium2 (Bass/Tile), 8-core SPMD.

Reference computation (per (b,t) slice, x:[N,F]):
    Q = x@Wq + bq; K = x@Wk + bk; V = x@Wv + bv
    s = Q @ K.T / sqrt(F)            # [N,N]
    a = softmax(s, axis=-1)
    out = diag(a)[:, None] * V       # only the softmax diagonal is needed

Sharding: data-parallel on the fused B*T axis (48 slices -> 6 per core),
weights replicated.

Fast path (zero biases; host folds M = Wq @ Wk^T so the K projection is
eliminated; scores = (X M) X^T):
  - x cast to fp16 on host; per slice, a single XBAR DMA-transpose loads
    X^T [f, n] straight from HBM (no PE transposes at all)
  - XMT[f',n] = M^T X^T in fp16 (PSUM fp32), copied to fp16 (diag path)
    and quantized to fp8e4 (scores path)
  - scores = XMT^T X^T in fp8e4 with DoubleRow perf mode (2 contraction
    rows/cycle = 4x fp32r rate).  Only the softmax DENOMINATOR comes from
    fp8 (row-sum of exp averages ~1024 terms, quantization noise cancels
    to ~0.1%); the NUMERATOR exp(s_nn) comes from an exact fp16 diagonal
    128x128 block matmul per row chunk.
  - V = X Wv in fp16, scaled by diag straight from PSUM
  - no max-subtraction: scaled scores are ~N(0,1), exp cannot overflow

Engine split per chunk: PE matmuls; ScalarE exp(+row-sum) and V scale-out;
DVE XMT fp16 copy, diag extract, softmax stats; Pool fp8 quantizations;
SP queue input DMA-transposes; Act queue output DMAs.
"""

import numpy as np

B, T, N, F = 4, 12, 1024, 512
NCORES = 8
S = (B * T) // NCORES  # 6 slices per core
P = 128
NO = N // P   # 8 row chunks per slice
FO = F // P   # 4 f chunks
SCALE = float(1.0 / np.sqrt(np.float32(F)))

_CACHE: dict = {}


def build_program(
    n_slices: int = S,
    repeats: int = 1,
    xt_bufs: int = 2,
    n_tr_splits: int = 1,
    split_cast: bool = False,
    enn_late: bool = False,
    outscale_act: bool = False,
    swap_copies: bool = False,
    dscr_bufs: int = 2,
    stats_bufs: int = 8,
    outp_bufs: int = 3,
    upto: int = 6,
    use_ttr: bool = False,
):
    import concourse.bass as bass
    import concourse.tile as tile
    from concourse import bacc, mybir
    from concourse.masks import make_identity
    from contextlib import ExitStack

    f32 = mybir.dt.float32
    f16 = mybir.dt.float16
    f8 = mybir.dt.float8e4
    DR = mybir.MatmulPerfMode.DoubleRow
    EXP = mybir.ActivationFunctionType.Exp
    COPYF = mybir.ActivationFunctionType.Identity
    OP = mybir.AluOpType
    AX = mybir.AxisListType.X

    nc = bacc.Bacc(trn_type="TRN2", target_bir_lowering=False, debug=False)
    x_d = nc.dram_tensor("x", [n_slices, N, F], f16, kind="ExternalInput").ap()
    m_d = nc.dram_tensor("m", [F, F], f16, kind="ExternalInput").ap()
    wv_d = nc.dram_tensor("wv", [F, F], f16, kind="ExternalInput").ap()
    out_d = nc.dram_tensor("out", [n_slices, N, F], f32, kind="ExternalOutput").ap()

    with tile.TileContext(nc) as tc, ExitStack() as ctx:
        consts = ctx.enter_context(tc.tile_pool(name="consts", bufs=1))
        xt_pool = ctx.enter_context(tc.tile_pool(name="xt", bufs=xt_bufs))
        xt8_pool = ctx.enter_context(tc.tile_pool(name="xt8", bufs=2))
        xmt_pool = ctx.enter_context(tc.tile_pool(name="xmt", bufs=2))
        xmt8_pool = ctx.enter_context(tc.tile_pool(name="xmt8", bufs=2))
        outp = ctx.enter_context(tc.tile_pool(name="outp", bufs=outp_bufs))
        dscr = ctx.enter_context(tc.tile_pool(name="dscr", bufs=dscr_bufs))
        stats = ctx.enter_context(tc.tile_pool(name="stats", bufs=stats_bufs))
        # PSUM budget: 8 banks = sp(2 tags x 2 bufs: scores ps0/ps1)
        # + pp(1 tag x 2: XMT halves and diag blocks) + pv(1 tag x 2: V)
        sp = ctx.enter_context(tc.tile_pool(name="sp", bufs=2, space="PSUM"))
        pp = ctx.enter_context(tc.tile_pool(name="pp", bufs=2, space="PSUM"))
        pv = ctx.enter_context(tc.tile_pool(name="pv", bufs=2, space="PSUM"))

        ident = consts.tile([P, P], f32, name="ident", tag="ident")
        make_identity(nc, ident[:])

        # weights: M = Wq @ Wk^T (host-folded) and Wv, both fp16,
        # [fi, fo, g] layout (f = fo*128 + fi on partitions)
        m_sb = consts.tile([P, FO, F], f16, name="m_sb", tag="m_sb")
        nc.sync.dma_start(m_sb[:], m_d.rearrange("(fo fi) g -> fi fo g", fi=P))
        wv_sb = consts.tile([P, FO, F], f16, name="wv_sb", tag="wv_sb")
        nc.sync.dma_start(wv_sb[:], wv_d.rearrange("(fo fi) g -> fi fo g", fi=P))

        slice_list = [sl for _ in range(repeats) for sl in range(n_slices)]
        for s in slice_list:
            # ---- X^T via XBAR DMA-transpose straight from HBM ----
            # xt[p, q, n] = x[s, n, q*128+p]  (f = q*128 + p)
            # split into n-quarters: finer DMA interleaving, and XMT's first
            # half can start after two quarters land
            xt = xt_pool.tile([P, FO, N], f16, name="xt", tag="xt")
            nsz = N // n_tr_splits
            for nq in range(n_tr_splits):
                nc.sync.dma_start_transpose(
                    xt[:, :, nq * nsz : (nq + 1) * nsz],
                    x_d[s, nq * nsz : (nq + 1) * nsz, :],
                )
            xt8 = xt8_pool.tile([P, FO, N], f8, name="xt8", tag="xt8")
            if split_cast:
                nc.gpsimd.tensor_copy(xt8[:, :, 0:512], xt[:, :, 0:512])
                nc.gpsimd.tensor_copy(xt8[:, :, 512:1024], xt[:, :, 512:1024])
            else:
                nc.gpsimd.tensor_copy(xt8[:], xt[:])

            if upto < 2:
                ot = outp.tile([P, F], f32, name="ot", tag="ot")
                nc.vector.tensor_copy(ot[:], xt[:, 0, 0:512])
                nc.sync.dma_start(out_d[s, 0:P, :], ot[:])
                continue

            # ---- XMT[f',n] = sum_f M[f,f'] XT[f,n]  (fp16, PSUM fp32) ----
            xmtb = xmt_pool.tile([P, FO, N], f16, name="xmtb", tag="xmtb")
            xmt8 = xmt8_pool.tile([P, FO, N], f8, name="xmt8", tag="xmt8")
            for b in range(FO):
                for h in range(2):
                    ps = pp.tile([P, 512], f32, name="ps_a", tag="ps_a")
                    for a in range(FO):
                        nc.tensor.matmul(
                            ps[:],
                            m_sb[:, a, b * P : (b + 1) * P],
                            xt[:, a, h * 512 : (h + 1) * 512],
                            start=(a == 0), stop=(a == FO - 1),
                        )
                    dst = xmtb[:, b, h * 512 : (h + 1) * 512]
                    dst8 = xmt8[:, b, h * 512 : (h + 1) * 512]
                    if swap_copies:
                        nc.gpsimd.tensor_copy(dst, ps[:])
                        nc.vector.tensor_copy(dst8, dst)
                    else:
                        nc.vector.tensor_copy(dst, ps[:])
                        nc.gpsimd.tensor_copy(dst8, dst)

            if upto < 3:
                ot = outp.tile([P, F], f32, name="ot", tag="ot")
                nc.vector.tensor_copy(ot[:], xmtb[:, 0, 0:512])
                nc.sync.dma_start(out_d[s, 0:P, :], ot[:])
                continue

            # ---- exact softmax numerators for all row chunks (fp16) ----
            # diag-block matmuls right after XMT, one TTR each, then a
            # single batched exp for the slice
            snn_all = stats.tile([P, NO], f32, name="snn_all", tag="snn_all")
            for no in range(NO):
                nlo, nhi = no * P, (no + 1) * P
                ps_d = pp.tile([P, 512], f32, name="ps_a", tag="ps_a")
                for go in range(FO):
                    nc.tensor.matmul(
                        ps_d[:, 0:P],
                        xmtb[:, go, nlo:nhi],
                        xt[:, go, nlo:nhi],
                        start=(go == 0), stop=(go == FO - 1),
                    )
                dblk = dscr.tile([P, P], f32, name="dblk", tag="dblk")
                if use_ttr:
                    nc.vector.tensor_tensor_reduce(
                        out=dblk[:],
                        in0=ps_d[:, 0:P],
                        in1=ident[:],
                        scale=1.0,
                        scalar=0.0,
                        op0=OP.mult,
                        op1=OP.add,
                        accum_out=snn_all[:, no : no + 1],
                    )
                else:
                    nc.vector.tensor_mul(dblk[:], ps_d[:, 0:P], ident[:])
                    nc.vector.tensor_reduce(
                        snn_all[:, no : no + 1], dblk[:], axis=AX, op=OP.add
                    )
            enn_all = stats.tile([P, NO], f32, name="enn_all", tag="enn_all")
            if not enn_late:
                nc.scalar.activation(enn_all[:], snn_all[:], EXP, scale=SCALE)

            if upto < 4:
                ot = outp.tile([P, F], f32, name="ot", tag="ot")
                nc.vector.tensor_scalar_mul(
                    ot[:], xmtb[:, 0, 0:512], enn_all[:, 0:1]
                )
                nc.sync.dma_start(out_d[s, 0:P, :], ot[:])
                continue

            # ---- per row chunk: V, fp8 scores, combine ----
            for no in range(NO):
                nlo, nhi = no * P, (no + 1) * P

                ps_v = pv.tile([P, F], f32, name="ps_v", tag="ps_v")
                for fo in range(FO):
                    nc.tensor.matmul(
                        ps_v[:],
                        xt[:, fo, nlo:nhi],
                        wv_sb[:, fo, :],
                        start=(fo == 0), stop=(fo == FO - 1),
                    )

                if upto < 5:
                    ot = outp.tile([P, F], f32, name="ot", tag="ot")
                    nc.vector.tensor_copy(ot[:], ps_v[:])
                    nc.sync.dma_start(out_d[s, nlo:nhi, :], ot[:])
                    continue

                # fp8 DoubleRow scores, one 2-bank PSUM tile, two groups
                ps_s = sp.tile([P, N], f32, name="ps_s", tag="ps_s")
                for q in range(2):
                    lhsT = xmt8[:, 2 * q : 2 * q + 2, nlo:nhi]
                    nc.tensor.matmul(
                        ps_s[:, 0:512], lhsT, xt8[:, 2 * q : 2 * q + 2, 0:512],
                        start=(q == 0), stop=(q == 1), perf_mode=DR,
                    )
                    nc.tensor.matmul(
                        ps_s[:, 512:1024], lhsT, xt8[:, 2 * q : 2 * q + 2, 512:1024],
                        start=(q == 0), stop=(q == 1), perf_mode=DR,
                    )

                # denominator: one exp over the full row (scale folded in),
                # row-sum via accum_out
                if upto < 6:
                    ot = outp.tile([P, F], f32, name="ot", tag="ot")
                    nc.vector.tensor_copy(ot[:], ps_s[:, 0:512])
                    nc.sync.dma_start(out_d[s, nlo:nhi, :], ot[:])
                    continue

                ssum = stats.tile([P, 1], f32, name="ssum", tag="ssum")
                nc.scalar.activation(
                    ps_s[:], ps_s[:], EXP, scale=SCALE, accum_out=ssum[:]
                )
                if enn_late and no == 0:
                    nc.scalar.activation(enn_all[:], snn_all[:], EXP, scale=SCALE)
                rec = stats.tile([P, 1], f32, name="rec", tag="rec")
                nc.vector.reciprocal(rec[:], ssum[:])
                dval = stats.tile([P, 1], f32, name="dval", tag="dval")
                nc.vector.tensor_mul(dval[:], enn_all[:, no : no + 1], rec[:])

                ot = outp.tile([P, F], f32, name="ot", tag="ot")
                if outscale_act:
                    nc.scalar.activation(ot[:], ps_v[:], COPYF, scale=dval[:])
                else:
                    nc.vector.tensor_scalar_mul(ot[:], ps_v[:], dval[:])
                nc.scalar.dma_start(out_d[s, nlo:nhi, :], ot[:])

    nc.compile()
    return nc


def _get_runner():
    """Build the Bass program once and wrap it in a cached jitted shard_map
    dispatcher (mirrors bass2jax.run_bass_via_pjrt, minus donation so the
    pre-zeroed output operands can be reused across calls — this kernel
    writes every output element)."""
    key = "runner"
    if key in _CACHE:
        return _CACHE[key]

    import jax
    from jax.experimental.shard_map import shard_map
    from jax.sharding import Mesh, NamedSharding, PartitionSpec
    from concourse import mybir
    from concourse.bass2jax import (
        _bass_exec_p,
        install_neuronx_cc_hook,
        partition_id_tensor,
    )

    nc = build_program(S)
    install_neuronx_cc_hook()
    partition_name = nc.partition_id_tensor.name if nc.partition_id_tensor else None

    in_names, out_names, out_avals, zero_outs = [], [], [], []
    for alloc in nc.m.functions[0].allocations:
        if not isinstance(alloc, mybir.MemoryLocationSet):
            continue
        name = alloc.memorylocations[0].name
        if alloc.kind == "ExternalInput":
            if name != partition_name:
                in_names.append(name)
        elif alloc.kind == "ExternalOutput":
            shape = tuple(alloc.tensor_shape)
            np_dt = mybir.dt.np(alloc.dtype)
            out_avals.append(jax.core.ShapedArray(shape, np_dt))
            out_names.append(name)
            zero_outs.append(np.zeros(shape, np_dt))

    n_params = len(in_names)
    all_in_names = list(in_names) + list(out_names)
    if partition_name is not None:
        all_in_names.append(partition_name)

    def _body(*args):
        operands = list(args)
        if partition_name is not None:
            operands.append(partition_id_tensor())
        outs = _bass_exec_p.bind(
            *operands,
            out_avals=tuple(out_avals),
            in_names=tuple(all_in_names),
            out_names=tuple(out_names),
            lowering_input_output_aliases=(),
            sim_require_finite=True,
            sim_require_nnan=True,
            nc=nc,
        )
        return tuple(outs)

    devices = jax.devices()[:NCORES]
    mesh = Mesh(np.asarray(devices), ("core",))
    n_outs = len(out_names)
    fn = jax.jit(
        shard_map(
            _body,
            mesh=mesh,
            in_specs=(PartitionSpec("core"),) * (n_params + n_outs),
            out_specs=(PartitionSpec("core"),) * n_outs,
            check_rep=False,
        ),
        keep_unused=True,
    )
    sharding = NamedSharding(mesh, PartitionSpec("core"))
    concat_zeros = [
        jax.device_put(
            np.zeros((NCORES * z.shape[0], *z.shape[1:]), z.dtype), sharding
        )
        for z in zero_outs
    ]
    runner = {
        "fn": fn,
        "in_names": in_names,
        "out_names": out_names,
        "zeros": concat_zeros,
        "sharding": sharding,
    }
    _CACHE[key] = runner
    return runner


def prep_inputs(x, Wq, Wk, Wv):
    """Host prep: fold M = Wq @ Wk^T, cast to fp16, tile weights per core.
    Returns the concatenated per-core input arrays keyed by dram tensor name."""
    f16 = np.float16
    x16 = np.ascontiguousarray(
        np.asarray(x, np.float32).reshape(NCORES * S, N, F).astype(f16)
    )
    M = np.ascontiguousarray(
        (np.asarray(Wq, np.float32) @ np.asarray(Wk, np.float32).T).astype(f16)
    )
    wv16 = np.ascontiguousarray(np.asarray(Wv, np.float32).astype(f16))
    return {
        "x": x16,
        "m": np.tile(M[None], (NCORES, 1, 1)).reshape(NCORES * F, F),
        "wv": np.tile(wv16[None], (NCORES, 1, 1)).reshape(NCORES * F, F),
    }


def kernel(x, Wq, bq, Wk, bk, Wv, bv):
    import jax

    x = np.asarray(x, dtype=np.float32)
    bq = np.asarray(bq, dtype=np.float32)
    bk = np.asarray(bk, dtype=np.float32)
    bv = np.asarray(bv, dtype=np.float32)
    Wq = np.asarray(Wq, dtype=np.float32)
    Wk = np.asarray(Wk, dtype=np.float32)
    Wv = np.asarray(Wv, dtype=np.float32)

    if bq.any() or bk.any() or bv.any():
        # general-bias fallback (never taken for the reference inputs, which
        # have zero biases): plain numpy evaluation of the reference formula
        Q = x @ Wq + bq
        K = x @ Wk + bk
        V = x @ Wv + bv
        s = np.einsum("btnf,btmf->btnm", Q, K) / np.sqrt(np.float32(F))
        s -= s.max(axis=-1, keepdims=True)
        e = np.exp(s)
        a = e / e.sum(axis=-1, keepdims=True)
        d = np.einsum("btnn->btn", a)
        return (d[..., None] * V).astype(np.float32)

    runner = _get_runner()
    per_core = prep_inputs(x, Wq, Wk, Wv)

    def _run(r):
        args = [
            jax.device_put(np.ascontiguousarray(per_core[nm]), r["sharding"])
            for nm in r["in_names"]
        ]
        outs = r["fn"](*args, *r["zeros"])
        return np.asarray(outs[r["out_names"].index("out")])

    try:
        out = _run(runner)
    except Exception:
        # stale cached executable/buffers (e.g. device session reset
        # between calls): rebuild once and retry
        _CACHE.pop("runner", None)
        out = _run(_get_runner())
    return out.reshape(B, T, N, F)


# revision 45
# speedup vs baseline: 1.3094x; 1.3094x over previous
"""GAT-style attention-diagonal kernel for Trainium2 (Bass/Tile), 8-core SPMD.

Reference computation (per (b,t) slice, x:[N,F]):
    Q = x@Wq + bq; K = x@Wk + bk; V = x@Wv + bv
    s = Q @ K.T / sqrt(F)            # [N,N]
    a = softmax(s, axis=-1)
    out = diag(a)[:, None] * V       # only the softmax diagonal is needed

Sharding: data-parallel on the fused B*T axis (48 slices -> 6 per core),
weights replicated.

Fast path (zero biases; host folds M = Wq @ Wk^T so the K projection is
eliminated; scores = (X M) X^T):
  - x cast to fp16 on host; per slice, a single XBAR DMA-transpose loads
    X^T [f, n] straight from HBM (no PE transposes at all)
  - XMT[f',n] = M^T X^T in fp16 (PSUM fp32), copied to fp16 (diag path)
    and quantized to fp8e4 (scores path)
  - scores = XMT^T X^T in fp8e4 with DoubleRow perf mode (2 contraction
    rows/cycle = 4x fp32r rate).  Only the softmax DENOMINATOR comes from
    fp8 (row-sum of exp averages ~1024 terms, quantization noise cancels
    to ~0.1%); the NUMERATOR exp(s_nn) comes from an exact fp16 diagonal
    128x128 block matmul per row chunk.
  - V = X Wv in fp16, scaled by diag straight from PSUM
  - no max-subtraction: scaled scores are ~N(0,1), exp cannot overflow

Engine split per chunk: PE matmuls; ScalarE exp(+row-sum) and V scale-out;
DVE XMT fp16 copy, diag extract, softmax stats; Pool fp8 quantizations;
SP queue input DMA-transposes; Act queue output DMAs.
"""

import numpy as np

B, T, N, F = 4, 12, 1024, 512
NCORES = 8
S = (B * T) // NCORES  # 6 slices per core
P = 128
NO = N // P   # 8 row chunks per slice
FO = F // P   # 4 f chunks
SCALE = float(1.0 / np.sqrt(np.float32(F)))

_CACHE: dict = {}


def build_program(
    n_slices: int = S,
    repeats: int = 1,
    xt_bufs: int = 2,
    n_tr_splits: int = 1,
    split_cast: bool = False,
    enn_late: bool = False,
    outscale_act: bool = False,
    swap_copies: bool = False,
    dscr_bufs: int = 2,
    stats_bufs: int = 8,
    outp_bufs: int = 4,
    upto: int = 6,
    use_ttr: bool = False,
    diag_interleaved: bool = False,
    xt8_on: str = 'dma',
    diag_paired: bool = True,
    outs_on: str = 'sync',
    outscale_on: str = 'dve',
    xmt_bufs: int = 2,
    diag_ex_on: str = 'dve',
):
    import concourse.bass as bass
    import concourse.tile as tile
    from concourse import bacc, mybir
    from concourse.masks import make_identity
    from contextlib import ExitStack

    f32 = mybir.dt.float32
    f16 = mybir.dt.float16
    f8 = mybir.dt.float8e4
    DR = mybir.MatmulPerfMode.DoubleRow
    EXP = mybir.ActivationFunctionType.Exp
    COPYF = mybir.ActivationFunctionType.Identity
    OP = mybir.AluOpType
    AX = mybir.AxisListType.X

    nc = bacc.Bacc(trn_type="TRN2", target_bir_lowering=False, debug=False)
    x_d = nc.dram_tensor("x", [n_slices, N, F], f16, kind="ExternalInput").ap()
    m_d = nc.dram_tensor("m", [F, F], f16, kind="ExternalInput").ap()
    wv_d = nc.dram_tensor("wv", [F, F], f16, kind="ExternalInput").ap()
    out_d = nc.dram_tensor("out", [n_slices, N, F], f32, kind="ExternalOutput").ap()

    with tile.TileContext(nc) as tc, ExitStack() as ctx:
        consts = ctx.enter_context(tc.tile_pool(name="consts", bufs=1))
        xt_pool = ctx.enter_context(tc.tile_pool(name="xt", bufs=xt_bufs))
        xt8_pool = ctx.enter_context(tc.tile_pool(name="xt8", bufs=2))
        xmt_pool = ctx.enter_context(tc.tile_pool(name="xmt", bufs=xmt_bufs))
        xmt8_pool = ctx.enter_context(tc.tile_pool(name="xmt8", bufs=xmt_bufs))
        outp = ctx.enter_context(tc.tile_pool(name="outp", bufs=outp_bufs))
        dscr = ctx.enter_context(tc.tile_pool(name="dscr", bufs=dscr_bufs))
        stats = ctx.enter_context(tc.tile_pool(name="stats", bufs=stats_bufs))
        # PSUM budget: 8 banks = sp(2 tags x 2 bufs: scores ps0/ps1)
        # + pp(1 tag x 2: XMT halves and diag blocks) + pv(1 tag x 2: V)
        sp = ctx.enter_context(tc.tile_pool(name="sp", bufs=2, space="PSUM"))
        pp = ctx.enter_context(tc.tile_pool(name="pp", bufs=2, space="PSUM"))
        pv = ctx.enter_context(tc.tile_pool(name="pv", bufs=2, space="PSUM"))

        ident = consts.tile([P, P], f32, name="ident", tag="ident")
        make_identity(nc, ident[:])
        ident2 = consts.tile([P, 2, P], f32, name="ident2", tag="ident2")
        nc.vector.tensor_copy(ident2[:, 0], ident[:])
        nc.vector.tensor_copy(ident2[:, 1], ident[:])

        # weights: M = Wq @ Wk^T (host-folded) and Wv, both fp16,
        # [fi, fo, g] layout (f = fo*128 + fi on partitions)
        m_sb = consts.tile([P, FO, F], f16, name="m_sb", tag="m_sb")
        nc.sync.dma_start(m_sb[:], m_d.rearrange("(fo fi) g -> fi fo g", fi=P))
        wv_sb = consts.tile([P, FO, F], f16, name="wv_sb", tag="wv_sb")
        nc.sync.dma_start(wv_sb[:], wv_d.rearrange("(fo fi) g -> fi fo g", fi=P))

        slice_list = [sl for _ in range(repeats) for sl in range(n_slices)]
        for s in slice_list:
            # ---- X^T via XBAR DMA-transpose straight from HBM ----
            # xt[p, q, n] = x[s, n, q*128+p]  (f = q*128 + p)
            # split into n-quarters: finer DMA interleaving, and XMT's first
            # half can start after two quarters land
            xt = xt_pool.tile([P, FO, N], f16, name="xt", tag="xt")
            nsz = N // n_tr_splits
            for nq in range(n_tr_splits):
                nc.sync.dma_start_transpose(
                    xt[:, :, nq * nsz : (nq + 1) * nsz],
                    x_d[s, nq * nsz : (nq + 1) * nsz, :],
                )
            xt8 = xt8_pool.tile([P, FO, N], f8, name="xt8", tag="xt8")
            if xt8_on == 'act':
                nc.scalar.activation(xt8[:], xt[:], COPYF)
            elif xt8_on == 'dma':
                nc.gpsimd.dma_start(xt8[:], xt[:])
            elif split_cast:
                nc.gpsimd.tensor_copy(xt8[:, :, 0:512], xt[:, :, 0:512])
                nc.gpsimd.tensor_copy(xt8[:, :, 512:1024], xt[:, :, 512:1024])
            else:
                nc.gpsimd.tensor_copy(xt8[:], xt[:])

            if upto < 2:
                ot = outp.tile([P, F], f32, name="ot", tag="ot")
                nc.vector.tensor_copy(ot[:], xt[:, 0, 0:512])
                nc.sync.dma_start(out_d[s, 0:P, :], ot[:])
                continue

            # ---- XMT[f',n] = sum_f M[f,f'] XT[f,n]  (fp16, PSUM fp32) ----
            xmtb = xmt_pool.tile([P, FO, N], f16, name="xmtb", tag="xmtb")
            xmt8 = xmt8_pool.tile([P, FO, N], f8, name="xmt8", tag="xmt8")
            for b in range(FO):
                for h in range(2):
                    ps = pp.tile([P, 512], f32, name="ps_a", tag="ps_a")
                    for a in range(FO):
                        nc.tensor.matmul(
                            ps[:],
                            m_sb[:, a, b * P : (b + 1) * P],
                            xt[:, a, h * 512 : (h + 1) * 512],
                            start=(a == 0), stop=(a == FO - 1),
                        )
                    dst = xmtb[:, b, h * 512 : (h + 1) * 512]
                    dst8 = xmt8[:, b, h * 512 : (h + 1) * 512]
                    if swap_copies:
                        nc.gpsimd.tensor_copy(dst, ps[:])
                        nc.vector.tensor_copy(dst8, dst)
                    else:
                        nc.vector.tensor_copy(dst, ps[:])
                        nc.gpsimd.tensor_copy(dst8, dst)

            if upto < 3:
                ot = outp.tile([P, F], f32, name="ot", tag="ot")
                nc.vector.tensor_copy(ot[:], xmtb[:, 0, 0:512])
                nc.sync.dma_start(out_d[s, 0:P, :], ot[:])
                continue

            # ---- exact softmax numerators for all row chunks (fp16) ----
            # diag-block matmuls right after XMT, one TTR each, then a
            # single batched exp for the slice
            snn_all = stats.tile([P, NO], f32, name="snn_all", tag="snn_all")
            if diag_paired and not diag_interleaved:
                for np_ in range(NO // 2):
                    ps_d = pp.tile([P, 512], f32, name="ps_a", tag="ps_a")
                    for half in range(2):
                        no = 2 * np_ + half
                        nlo, nhi = no * P, (no + 1) * P
                        for go in range(FO):
                            nc.tensor.matmul(
                                ps_d[:, half * P : (half + 1) * P],
                                xmtb[:, go, nlo:nhi],
                                xt[:, go, nlo:nhi],
                                start=(go == 0), stop=(go == FO - 1),
                            )
                    dblk = dscr.tile([P, 2, P], f32, name="dblk", tag="dblk")
                    deng = nc.gpsimd if diag_ex_on == 'pool' else nc.vector
                    deng.tensor_mul(
                        dblk[:],
                        ps_d[:, 0:256].rearrange("p (two q) -> p two q", two=2),
                        ident2[:],
                    )
                    deng.tensor_reduce(
                        snn_all[:, 2 * np_ : 2 * np_ + 2],
                        dblk[:],
                        axis=AX,
                        op=OP.add,
                    )
            for no in ([] if (diag_interleaved or diag_paired) else range(NO)):
                nlo, nhi = no * P, (no + 1) * P
                ps_d = pp.tile([P, 512], f32, name="ps_a", tag="ps_a")
                for go in range(FO):
                    nc.tensor.matmul(
                        ps_d[:, 0:P],
                        xmtb[:, go, nlo:nhi],
                        xt[:, go, nlo:nhi],
                        start=(go == 0), stop=(go == FO - 1),
                    )
                dblk = dscr.tile([P, P], f32, name="dblk", tag="dblk")
                if use_ttr:
                    nc.vector.tensor_tensor_reduce(
                        out=dblk[:],
                        in0=ps_d[:, 0:P],
                        in1=ident[:],
                        scale=1.0,
                        scalar=0.0,
                        op0=OP.mult,
                        op1=OP.add,
                        accum_out=snn_all[:, no : no + 1],
                    )
                else:
                    nc.vector.tensor_mul(dblk[:], ps_d[:, 0:P], ident[:])
                    nc.vector.tensor_reduce(
                        snn_all[:, no : no + 1], dblk[:], axis=AX, op=OP.add
                    )
            enn_all = stats.tile([P, NO], f32, name="enn_all", tag="enn_all")
            if not enn_late and not diag_interleaved:
                nc.scalar.activation(enn_all[:], snn_all[:], EXP, scale=SCALE)

            if upto < 4:
                ot = outp.tile([P, F], f32, name="ot", tag="ot")
                nc.vector.tensor_scalar_mul(
                    ot[:], xmtb[:, 0, 0:512], enn_all[:, 0:1]
                )
                nc.sync.dma_start(out_d[s, 0:P, :], ot[:])
                continue

            # ---- per row chunk: V, fp8 scores, combine ----
            for no in range(NO):
                nlo, nhi = no * P, (no + 1) * P

                if diag_interleaved:
                    ps_d = pp.tile([P, 512], f32, name="ps_a", tag="ps_a")
                    for go in range(FO):
                        nc.tensor.matmul(
                            ps_d[:, 0:P],
                            xmtb[:, go, nlo:nhi],
                            xt[:, go, nlo:nhi],
                            start=(go == 0), stop=(go == FO - 1),
                        )
                    dblk = dscr.tile([P, P], f32, name="dblk", tag="dblk")
                    nc.vector.tensor_mul(dblk[:], ps_d[:, 0:P], ident[:])
                    nc.vector.tensor_reduce(
                        snn_all[:, no : no + 1], dblk[:], axis=AX, op=OP.add
                    )
                    nc.scalar.activation(
                        enn_all[:, no : no + 1], snn_all[:, no : no + 1],
                        EXP, scale=SCALE,
                    )

                ps_v = pv.tile([P, F], f32, name="ps_v", tag="ps_v")
                for fo in range(FO):
                    nc.tensor.matmul(
                        ps_v[:],
                        xt[:, fo, nlo:nhi],
                        wv_sb[:, fo, :],
                        start=(fo == 0), stop=(fo == FO - 1),
                    )

                if upto < 5:
                    ot = outp.tile([P, F], f32, name="ot", tag="ot")
                    nc.vector.tensor_copy(ot[:], ps_v[:])
                    nc.sync.dma_start(out_d[s, nlo:nhi, :], ot[:])
                    continue

                # fp8 DoubleRow scores, one 2-bank PSUM tile, two groups
                ps_s = sp.tile([P, N], f32, name="ps_s", tag="ps_s")
                for q in range(2):
                    lhsT = xmt8[:, 2 * q : 2 * q + 2, nlo:nhi]
                    nc.tensor.matmul(
                        ps_s[:, 0:512], lhsT, xt8[:, 2 * q : 2 * q + 2, 0:512],
                        start=(q == 0), stop=(q == 1), perf_mode=DR,
                    )
                    nc.tensor.matmul(
                        ps_s[:, 512:1024], lhsT, xt8[:, 2 * q : 2 * q + 2, 512:1024],
                        start=(q == 0), stop=(q == 1), perf_mode=DR,
                    )

                # denominator: one exp over the full row (scale folded in),
                # row-sum via accum_out
                if upto < 6:
                    ot = outp.tile([P, F], f32, name="ot", tag="ot")
                    nc.vector.tensor_copy(ot[:], ps_s[:, 0:512])
                    nc.sync.dma_start(out_d[s, nlo:nhi, :], ot[:])
                    continue

                ssum = stats.tile([P, 1], f32, name="ssum", tag="ssum")
                nc.scalar.activation(
                    ps_s[:], ps_s[:], EXP, scale=SCALE, accum_out=ssum[:]
                )
                if enn_late and no == 0:
                    nc.scalar.activation(enn_all[:], snn_all[:], EXP, scale=SCALE)
                rec = stats.tile([P, 1], f32, name="rec", tag="rec")
                nc.vector.reciprocal(rec[:], ssum[:])
                dval = stats.tile([P, 1], f32, name="dval", tag="dval")
                nc.vector.tensor_mul(dval[:], enn_all[:, no : no + 1], rec[:])

                ot = outp.tile([P, F], f32, name="ot", tag="ot")
                if outscale_on == 'act' or outscale_act:
                    nc.scalar.activation(ot[:], ps_v[:], COPYF, scale=dval[:])
                elif outscale_on == 'pool':
                    nc.gpsimd.tensor_scalar_mul(ot[:], ps_v[:], dval[:])
                else:
                    nc.vector.tensor_scalar_mul(ot[:], ps_v[:], dval[:])
                out_eng = {"scalar": nc.scalar, "sync": nc.sync, "gpsimd": nc.gpsimd}[outs_on]
                out_eng.dma_start(out_d[s, nlo:nhi, :], ot[:])

    nc.compile()
    return nc


def _get_runner():
    """Build the Bass program once and wrap it in a cached jitted shard_map
    dispatcher (mirrors bass2jax.run_bass_via_pjrt, minus donation so the
    pre-zeroed output operands can be reused across calls — this kernel
    writes every output element)."""
    key = "runner"
    if key in _CACHE:
        return _CACHE[key]

    import jax
    from jax.experimental.shard_map import shard_map
    from jax.sharding import Mesh, NamedSharding, PartitionSpec
    from concourse import mybir
    from concourse.bass2jax import (
        _bass_exec_p,
        install_neuronx_cc_hook,
        partition_id_tensor,
    )

    nc = build_program(S)
    install_neuronx_cc_hook()
    partition_name = nc.partition_id_tensor.name if nc.partition_id_tensor else None

    in_names, out_names, out_avals, zero_outs = [], [], [], []
    for alloc in nc.m.functions[0].allocations:
        if not isinstance(alloc, mybir.MemoryLocationSet):
            continue
        name = alloc.memorylocations[0].name
        if alloc.kind == "ExternalInput":
            if name != partition_name:
                in_names.append(name)
        elif alloc.kind == "ExternalOutput":
            shape = tuple(alloc.tensor_shape)
            np_dt = mybir.dt.np(alloc.dtype)
            out_avals.append(jax.core.ShapedArray(shape, np_dt))
            out_names.append(name)
            zero_outs.append(np.zeros(shape, np_dt))

    n_params = len(in_names)
    all_in_names = list(in_names) + list(out_names)
    if partition_name is not None:
        all_in_names.append(partition_name)

    def _body(*args):
        operands = list(args)
        if partition_name is not None:
            operands.append(partition_id_tensor())
        outs = _bass_exec_p.bind(
            *operands,
            out_avals=tuple(out_avals),
            in_names=tuple(all_in_names),
            out_names=tuple(out_names),
            lowering_input_output_aliases=(),
            sim_require_finite=True,
            sim_require_nnan=True,
            nc=nc,
        )
        return tuple(outs)

    devices = jax.devices()[:NCORES]
    mesh = Mesh(np.asarray(devices), ("core",))
    n_outs = len(out_names)
    fn = jax.jit(
        shard_map(
            _body,
            mesh=mesh,
            in_specs=(PartitionSpec("core"),) * (n_params + n_outs),
            out_specs=(PartitionSpec("core"),) * n_outs,
            check_rep=False,
        ),
        keep_unused=True,
    )
    sharding = NamedSharding(mesh, PartitionSpec("core"))
    concat_zeros = [
        jax.device_put(
            np.zeros((NCORES * z.shape[0], *z.shape[1:]), z.dtype), sharding
        )
        for z in zero_outs
    ]
    runner = {
        "fn": fn,
        "in_names": in_names,
        "out_names": out_names,
        "zeros": concat_zeros,
        "sharding": sharding,
    }
    _CACHE[key] = runner
    return runner


def prep_inputs(x, Wq, Wk, Wv):
    """Host prep: fold M = Wq @ Wk^T, cast to fp16, tile weights per core.
    Returns the concatenated per-core input arrays keyed by dram tensor name."""
    f16 = np.float16
    x16 = np.ascontiguousarray(
        np.asarray(x, np.float32).reshape(NCORES * S, N, F).astype(f16)
    )
    M = np.ascontiguousarray(
        (np.asarray(Wq, np.float32) @ np.asarray(Wk, np.float32).T).astype(f16)
    )
    wv16 = np.ascontiguousarray(np.asarray(Wv, np.float32).astype(f16))
    return {
        "x": x16,
        "m": np.tile(M[None], (NCORES, 1, 1)).reshape(NCORES * F, F),
        "wv": np.tile(wv16[None], (NCORES, 1, 1)).reshape(NCORES * F, F),
    }


def kernel(x, Wq, bq, Wk, bk, Wv, bv):
    import jax

    x = np.asarray(x, dtype=np.float32)
    bq = np.asarray(bq, dtype=np.float32)
    bk = np.asarray(bk, dtype=np.float32)
    bv = np.asarray(bv, dtype=np.float32)
    Wq = np.asarray(Wq, dtype=np.float32)
    Wk = np.asarray(Wk, dtype=np.float32)
    Wv = np.asarray(Wv, dtype=np.float32)

    if bq.any() or bk.any() or bv.any():
        # general-bias fallback (never taken for the reference inputs, which
        # have zero biases): plain numpy evaluation of the reference formula
        Q = x @ Wq + bq
        K = x @ Wk + bk
        V = x @ Wv + bv
        s = np.einsum("btnf,btmf->btnm", Q, K) / np.sqrt(np.float32(F))
        s -= s.max(axis=-1, keepdims=True)
        e = np.exp(s)
        a = e / e.sum(axis=-1, keepdims=True)
        d = np.einsum("btnn->btn", a)
        return (d[..., None] * V).astype(np.float32)

    runner = _get_runner()
    per_core = prep_inputs(x, Wq, Wk, Wv)

    def _run(r):
        args = [
            jax.device_put(np.ascontiguousarray(per_core[nm]), r["sharding"])
            for nm in r["in_names"]
        ]
        outs = r["fn"](*args, *r["zeros"])
        return np.asarray(outs[r["out_names"].index("out")])

    try:
        out = _run(runner)
    except Exception:
        # stale cached executable/buffers (e.g. device session reset
        # between calls): rebuild once and retry
        _CACHE.pop("runner", None)
        out = _run(_get_runner())
    return out.reshape(B, T, N, F)
